# revision 2
# baseline (speedup 1.0000x reference)
"""Trainium2 Bass kernel for nn_BinaryDense: out = x @ (sum_k sign(b_k)*a_k) + bias.

Shapes (hardcoded): x [4096,4096] f32, b [4,4096,4096] f32, a [4,4096] f32,
bias [4096] f32 -> out [4096,4096] f32.

Strategy: tensor-parallel over the output (units) dim across 8 NeuronCores.
Core c owns O-columns [c*512, (c+1)*512).

Per core: one bf16 matmul x @ w with w built on-chip.
  w[:, oc] = sum_k copysign(a[k,oc], b[k,:,oc]); b arrives bf16 in
  [I, K, O_c] (k-major) layout. Build per 128-row k-tile is 3 DVE ops:
    contrib = (b & 0x80008000) | a   (one fused scalar_tensor_tensor, int32)
    t = contrib[0:2] + contrib[2:4]  (bf16 add, 1024 wide)
    w = t[0] + t[1]                  (bf16 add, 512 wide)

Schedule (pair-kt-major): 32 m-tiles in 8 mbs of 4; PSUM = two wide
[128, 2048] f32 tiles (4 banks each) holding a PAIR of mbs = 8 m-tiles in
flight. For each k-block (K_BLOCKS=[4,6,8,8,6]) the 4 pairs sweep the
block's k-tiles kt-major: each w tile gets 8 matmuls (1.7us) of PE work per
visit, so the DVE build (~1.85us/tile) only has to keep pace during the
first pair of kb0, which the HAM warm-up dummies absorb. Per-pair psum
eviction is a single wide DVE add into a wide fp32 SBUF accumulator
(bias folded into the kb0 evict). Final k-block evicts o = ps + acc and
stores via 4 narrow DMAs on the otherwise-idle Scalar engine queue.

DMA layout: xt tiles [128, 1024] (one per (kt, pair)) stream on the sync
HWDGE queue; b tiles stream on the GpSimd SWDGE queue so they are not
head-of-line blocked behind xt; consts + out stores ride Scalar/GpSimd.

Host side only reshapes/casts/shards (no math): x^T bf16, b -> [I,K,O] bf16,
a broadcast rows, bias tiled 4x to [128, 2048] f32.
"""

import sys

if "/opt/trn_rl_repo" not in sys.path:
    sys.path.insert(0, "/opt/trn_rl_repo")

import numpy as np
import ml_dtypes

BF16 = ml_dtypes.bfloat16

B = 4096   # batch rows of x
I = 4096   # input dim (contraction)
O = 4096   # output dim (sharded)
K = 4      # binary bases
NCORES = 8
OC = O // NCORES   # 512 output cols per core
P = 128

KT = I // P        # 32 k-tiles (contraction)
MT = B // P        # 32 m-tiles (output rows)

SIGNMASK = -2147450880  # 0x80008000: bf16 sign-bit pair as int32


def _build_program():
    import os
    import math
    import concourse.bass as bass
    import concourse.mybir as mybir
    from concourse import bacc
    from concourse.tile import TileContext

    nc = bacc.Bacc(None, target_bir_lowering=False)

    b_re = nc.declare_dram_parameter("b_re", [I, K * OC], mybir.dt.bfloat16, isOutput=False)
    a_b = nc.declare_dram_parameter("a_b", [P, K * OC], mybir.dt.bfloat16, isOutput=False)
    xT = nc.declare_dram_parameter("xT", [I, B], mybir.dt.bfloat16, isOutput=False)
    bias_w = nc.declare_dram_parameter("bias_w", [P, 4 * OC], mybir.dt.float32, isOutput=False)
    out = nc.declare_dram_parameter("out", [B, OC], mybir.dt.float32, isOutput=True)

    K_BLOCKS = [int(s) for s in os.environ.get("BK_KBLOCKS", "4,6,8,8,6").split(",")]
    assert sum(K_BLOCKS) == KT
    NKB = len(K_BLOCKS)
    k_starts = [sum(K_BLOCKS[:i]) for i in range(NKB)]
    N_DUM = int(os.environ.get("BK_DUMMIES", "24"))
    FUSED = os.environ.get("BK_FUSED", "1") == "1"

    with TileContext(nc) as tc:
        with (
            tc.tile_pool(name="const", bufs=1) as const,
            tc.tile_pool(name="bpool", bufs=6) as bpool,
            tc.tile_pool(name="cpool", bufs=3) as cpool,
            tc.tile_pool(name="tpool", bufs=3) as tpool,
            tc.tile_pool(name="wpool", bufs=1) as wpool,
            tc.tile_pool(name="xpool", bufs=8) as xpool,
            tc.tile_pool(name="apool", bufs=1) as apool,
            tc.tile_pool(name="opool", bufs=2) as opool,
            tc.tile_pool(name="psum", bufs=2, space="PSUM") as psum_pool,
        ):
            # consts on the gpsimd SWDGE queue (b tiles follow there)
            a_tile = const.tile([P, K * OC], mybir.dt.bfloat16)
            nc.gpsimd.dma_start(out=a_tile[:], in_=a_b[:, :])
            bias_tile = const.tile([P, 4 * OC], mybir.dt.float32)
            nc.gpsimd.dma_start(out=bias_tile[:], in_=bias_w[:, :])
            mask_tile = const.tile([P, 1], mybir.dt.int32)
            nc.vector.memset(mask_tile[:], SIGNMASK)
            dummy_w = const.tile([P, P], mybir.dt.bfloat16)
            nc.vector.memset(dummy_w[:], 0)
            dummy_rhs = const.tile([P, OC], mybir.dt.bfloat16)
            nc.vector.memset(dummy_rhs[:], 0)

            # ---- w-build machinery ----
            b_live = {}
            w_tiles = [None] * KT

            def emit_bdma(kt):
                b_tile = bpool.tile([P, K * OC], mybir.dt.bfloat16, name="b_tile")
                nc.gpsimd.dma_start(out=b_tile[:], in_=b_re[kt * P:(kt + 1) * P, :])
                b_live[kt] = b_tile

            def emit_build(kt):
                b_tile = b_live.pop(kt)
                contrib = cpool.tile([P, K * OC], mybir.dt.bfloat16, name="contrib")
                if FUSED:
                    nc.vector.scalar_tensor_tensor(
                        out=contrib.bitcast(mybir.dt.int32)[:],
                        in0=b_tile.bitcast(mybir.dt.int32)[:],
                        scalar=mask_tile[:, 0:1],
                        in1=a_tile.bitcast(mybir.dt.int32)[:],
                        op0=mybir.AluOpType.bitwise_and,
                        op1=mybir.AluOpType.bitwise_or,
                    )
                else:
                    nc.vector.tensor_scalar(
                        out=b_tile.bitcast(mybir.dt.int32)[:],
                        in0=b_tile.bitcast(mybir.dt.int32)[:],
                        scalar1=mask_tile[:, 0:1],
                        scalar2=None,
                        op0=mybir.AluOpType.bitwise_and,
                    )
                    nc.vector.tensor_tensor(
                        out=contrib.bitcast(mybir.dt.int16)[:],
                        in0=b_tile.bitcast(mybir.dt.int16)[:],
                        in1=a_tile.bitcast(mybir.dt.int16)[:],
                        op=mybir.AluOpType.bitwise_or,
                    )
                t_tile = tpool.tile([P, 2 * OC], mybir.dt.bfloat16, name="t_tile")
                nc.vector.tensor_tensor(
                    out=t_tile[:],
                    in0=contrib[:, 0:2 * OC],
                    in1=contrib[:, 2 * OC:4 * OC],
                    op=mybir.AluOpType.add,
                )
                w_tile = wpool.tile([P, OC], mybir.dt.bfloat16, name=f"w_{kt}")
                nc.vector.tensor_tensor(
                    out=w_tile[:],
                    in0=t_tile[:, 0:OC],
                    in1=t_tile[:, OC:2 * OC],
                    op=mybir.AluOpType.add,
                )
                w_tiles[kt] = w_tile

            def emit_xt(kt, pair):
                xt = xpool.tile([P, 8 * P], mybir.dt.bfloat16, name="xt")
                nc.sync.dma_start(
                    out=xt[:],
                    in_=xT[kt * P:(kt + 1) * P, pair * 8 * P:(pair + 1) * 8 * P],
                )
                return xt

            # ---- prime: kb0's b, pair0's xt, HAM dummies, kb0's builds ----
            for kt in range(K_BLOCKS[0]):
                emit_bdma(kt)
            xt_next = [emit_xt(kt, 0) for kt in range(k_starts[0], K_BLOCKS[0])]
            for _ in range(N_DUM):
                dps = psum_pool.tile([P, 4 * OC], mybir.dt.float32, name="psw")
                nc.tensor.matmul(dps[:, 0:OC], dummy_w[:], dummy_rhs[:],
                                 start=True, stop=True)
            for kt in range(K_BLOCKS[0]):
                emit_build(kt)
            build_cursor = K_BLOCKS[0]

            # ---- main pair-kt-major loop ----
            acc_t = {}
            for kb in range(NKB):
                k0, KB = k_starts[kb], K_BLOCKS[kb]
                for p in range(4):
                    mbA, mbB = 2 * p, 2 * p + 1
                    xt_cur = xt_next
                    if (kb, p) != (NKB - 1, 3):
                        nkb, np_ = (kb, p + 1) if p < 3 else (kb + 1, 0)
                        nk0, nKB = k_starts[nkb], K_BLOCKS[nkb]
                        xt_next = [emit_xt(kt, np_) for kt in range(nk0, nk0 + nKB)]
                    psA = psum_pool.tile([P, 4 * OC], mybir.dt.float32, name="psw")
                    psB = psum_pool.tile([P, 4 * OC], mybir.dt.float32, name="psw")
                    for i, kt in enumerate(range(k0, k0 + KB)):
                        xt = xt_cur[i]
                        for ps, mb in ((psA, mbA), (psB, mbB)):
                            base = (mb - 2 * p) * 4
                            for j in range(4):
                                loc = base + j
                                nc.tensor.matmul(
                                    ps[:, j * OC:(j + 1) * OC],
                                    xt[:, loc * P:(loc + 1) * P],
                                    w_tiles[kt][:],
                                    start=(i == 0),
                                    stop=(i == KB - 1),
                                )
                    # evicts (one wide DVE add per mb)
                    for ps, mb in ((psA, mbA), (psB, mbB)):
                        if kb == 0:
                            acc = apool.tile([P, 4 * OC], mybir.dt.float32,
                                             name=f"acc_{mb}")
                            nc.vector.tensor_tensor(
                                out=acc[:], in0=ps[:], in1=bias_tile[:],
                                op=mybir.AluOpType.add,
                            )
                            acc_t[mb] = acc
                        elif kb < NKB - 1:
                            acc = acc_t[mb]
                            nc.vector.tensor_tensor(
                                out=acc[:], in0=ps[:], in1=acc[:],
                                op=mybir.AluOpType.add,
                            )
                        else:
                            o_tile = opool.tile([P, 4 * OC], mybir.dt.float32,
                                                name="o_wide")
                            nc.vector.tensor_tensor(
                                out=o_tile[:], in0=ps[:], in1=acc_t[mb][:],
                                op=mybir.AluOpType.add,
                            )
                            for j in range(4):
                                m = mb * 4 + j
                                nc.scalar.dma_start(
                                    out=out[m * P:(m + 1) * P, :],
                                    in_=o_tile[:, j * OC:(j + 1) * OC],
                                )
                    # build-ahead for the next k-block
                    if kb + 1 < NKB:
                        target = k_starts[kb + 1] + math.ceil(
                            K_BLOCKS[kb + 1] * (p + 1) / 4)
                        while build_cursor < target:
                            emit_bdma(build_cursor)
                            emit_build(build_cursor)
                            build_cursor += 1

    nc.compile()
    return nc


_NC_CACHE = None


def _get_program():
    global _NC_CACHE
    if _NC_CACHE is None:
        _NC_CACHE = _build_program()
    return _NC_CACHE


def prep_inputs(x, b, a, bias):
    """Host-side shard/cast/layout only. Returns per-core input maps."""
    x = np.asarray(x, dtype=np.float32)
    b = np.asarray(b, dtype=np.float32)
    a = np.asarray(a, dtype=np.float32)
    bias = np.asarray(bias, dtype=np.float32)
    xT16 = np.ascontiguousarray(x.T).astype(BF16)          # [I, B] bf16
    b_iko = np.transpose(b, (1, 0, 2)).astype(BF16)        # [I, K, O] bf16
    bias32 = bias.astype(np.float32)
    a16 = a.astype(BF16)                                    # [K, O]

    in_maps = []
    for c in range(NCORES):
        sl = slice(c * OC, (c + 1) * OC)
        b_slice = np.ascontiguousarray(b_iko[:, :, sl]).reshape(I, K * OC)
        a_flat = np.ascontiguousarray(a16[:, sl]).reshape(1, K * OC)
        a_bcast = np.broadcast_to(a_flat, (P, K * OC)).copy()
        bias_wide = np.broadcast_to(
            np.tile(bias32[sl], 4).reshape(1, 4 * OC), (P, 4 * OC)).copy()
        in_maps.append({
            "b_re": b_slice,
            "a_b": a_bcast,
            "xT": xT16,
            "bias_w": bias_wide,
        })
    return in_maps


def run(in_maps, trace=False):
    from concourse.bass_utils import run_bass_kernel_spmd

    nc = _get_program()
    res = run_bass_kernel_spmd(nc, in_maps, list(range(NCORES)), trace=trace)
    return res


def kernel(x, b, a, bias):
    in_maps = prep_inputs(x, b, a, bias)
    res = run(in_maps)
    out = np.concatenate([res.results[c]["out"] for c in range(NCORES)], axis=1)
    return np.ascontiguousarray(out, dtype=np.float32)


if __name__ == "__main__":
    rng = np.random.default_rng(0)
    x = rng.standard_normal((B, I), dtype=np.float32)
    b = rng.standard_normal((K, I, O), dtype=np.float32)
    a = rng.random((K, O), dtype=np.float32)
    bias = rng.standard_normal(O, dtype=np.float32)
    out = kernel(x=x, b=b, a=a, bias=bias)
    w_eff = np.einsum('kio,ko->io', np.sign(b), a.astype(np.float64)).astype(np.float64)
    expected = x.astype(np.float64) @ w_eff + bias
    rel = np.linalg.norm(out - expected) / np.linalg.norm(expected)
    print(f"rel_err = {rel:.3e}")
